# revision 16
# baseline (speedup 1.0000x reference)
"""3x3 windowed mean-imputation (nn_Averager) on 8 trn2 NeuronCores.

out = where(|x| > 2.5, Wsum3x3(x*valid) / Wcnt3x3(valid), x)
valid = |x| < 2.5, SAME zero padding. (Wcnt >= 1 at every faulty point for
this input; |x| == 2.5 never occurs — both verified offline in test.py.)

Sharding: pure data parallel. x is (16, 64, 256, 256) fp32; each of the 8
cores gets 2 N slices = 128 images, laid out as [128 partitions, 65536 free]
(partition = image, free = flattened h*256+w). Both window axes are free-dim
shifts; vertical pass uses 1-row halos, horizontal edge columns are
overwritten with 2-term sums. Counts ride in bf16 (exact for ints <= 9).
"""

import sys

sys.path.insert(0, "/opt/trn_rl_repo")

import numpy as np

import concourse.bacc as bacc
import concourse.mybir as mybir
from concourse import bass_utils
from concourse.mybir import AluOpType
from concourse.tile import TileContext

N, C, H, W = 16, 64, 256, 256
NCORES = 8
P = (N // NCORES) * C  # 128 images per core = 128 partitions
FREE = H * W  # 65536
K = 16  # image rows per tile
R = K + 2  # with halo rows
E = R * W  # extended tile free size
KW = K * W
BIG = 1.0e9  # halo fill: |BIG| > 2.5 so vf=0 and x*vf=0

F32 = mybir.dt.float32
BF16 = mybir.dt.bfloat16
I32 = mybir.dt.int32

# s-path (windowed value sums) dtype: BF16 halves the shifted-add cost (2x
# DVE mode) at ~5e-3 rel err on imputed points; F32 keeps absmax err ~4e-6.
S_BF16 = False
S_DT = BF16 if S_BF16 else F32

# masks via int32 bitcast (|x| as sign-bit clear, compare in int domain);
# falls back to an |x| pass on ScalarE if disabled.
BITCAST_MASK = False
ABS_BITS_25 = 0x40200000  # bits(2.5f)
SIGN_CLEAR = 0x7FFFFFFF

_NC_CACHE = None


def _register_select_band():
    """Custom DVE op: out = select(in1 > s0 or in1 < s1, in0, in1).

    One 1x-rate Vector op replacing mask-gen + copy_predicated for the final
    blend (in0 = window mean, in1 = x)."""
    from concourse import dve_ops
    from concourse.dve_spec import C0, C1, Spec, Src0, Src1, _has_src1
    from concourse.dve_spec import lower as dve_lower
    from concourse.dve_spec import select as dve_select
    from concourse.dve_uop import DveOpSpec

    name = "SELECT_BAND_ANT"
    if name in dve_ops._SUB_OPCODE_FOR_NAME:
        return next(op for op in dve_ops.OPS if op.name == name)

    spec = Spec(
        body=dve_select((Src1 > C0) | (Src1 < C1), Src0, Src1),
        reference=lambda in0, in1, s0, s1, imm2: np.where(
            (in1 > s0) | (in1 < s1), in0, in1
        ).astype(np.float32),
    )
    row = max(dve_ops._SUB_OPCODE_FOR_NAME.values()) + 1
    assert row < 0x20
    dve_ops._SUB_OPCODE_FOR_NAME[name] = row
    try:
        shas = {}
        for ver in ("v3", "v4"):
            tmp = DveOpSpec(
                name=name,
                opcode=row,
                uops=dve_lower(spec, ver=ver),
                rd1_en=_has_src1(spec),
            )
            shas[ver] = tmp.sha(ver)
        op = dve_ops.DveOp(name, spec, subdim=False, uops_sha=shas)
    except Exception:
        del dve_ops._SUB_OPCODE_FOR_NAME[name]
        raise
    dve_ops.OPS.append(op)
    dve_ops.CUSTOM_DVE_SPECS[name] = spec
    return op


try:
    _SELECT_BAND = _register_select_band()
except Exception:
    _SELECT_BAND = None


def build_nc():
    nc = bacc.Bacc("TRN2", target_bir_lowering=False)
    # bias constant for the Relu(2.5 - |x|) activation
    _c = nc.alloc_sbuf_tensor("const-float32-2.5", [128, 1], F32)
    nc.gpsimd.memset(_c.ap(), 2.5)
    nc.const_aps.aps[(F32, 2.5)] = _c.ap()
    nc.all_engine_barrier()
    x = nc.dram_tensor("x", [P, FREE], F32, kind="ExternalInput")
    out = nc.dram_tensor("out", [P, FREE], F32, kind="ExternalOutput")

    with TileContext(nc) as tc:
        with (
            tc.tile_pool(name="io", bufs=2) as iop,
            tc.tile_pool(name="wk", bufs=1) as wk,
        ):
            n_tiles = H // K
            for t in range(n_tiles):
                xe = iop.tile([P, E], F32, tag="xe")
                # ---- load x rows [t*K-1, t*K+K+1) with BIG-filled halo at
                # image top/bottom (BIG -> invalid, contributes 0 to sums)
                if t == 0:
                    nc.vector.memset(xe[:, 0:W], BIG)
                    nc.sync.dma_start(xe[:, W:E], x[:, 0 : (K + 1) * W])
                elif t == n_tiles - 1:
                    nc.sync.dma_start(
                        xe[:, 0 : (K + 1) * W], x[:, (t * K - 1) * W : FREE]
                    )
                    nc.vector.memset(xe[:, (K + 1) * W : E], BIG)
                else:
                    nc.sync.dma_start(
                        xe[:, :], x[:, (t * K - 1) * W : (t * K + K + 1) * W]
                    )

                # ---- vf = (|x| < 2.5) as bf16 0/1
                vf = iop.tile([P, E], BF16, tag="vf")
                if BITCAST_MASK:
                    nc.vector.tensor_scalar(
                        vf[:, :],
                        xe[:, :].bitcast(I32),
                        SIGN_CLEAR,
                        ABS_BITS_25,
                        AluOpType.bitwise_and,
                        AluOpType.is_lt,
                    )
                else:
                    ab = wk.tile([P, E], F32, tag="ab")
                    nc.scalar.activation(
                        ab[:, :], xe[:, :], mybir.ActivationFunctionType.Abs
                    )
                    nc.scalar.activation(
                        ab[:, :],
                        ab[:, :],
                        mybir.ActivationFunctionType.Relu,
                        bias=2.5,
                        scale=-1.0,
                    )
                    nc.scalar.activation(
                        vf[:, :],
                        ab[:, :],
                        mybir.ActivationFunctionType.Tanh,
                        scale=1.0e38,
                    )
                # xv = x * vf  (valid values, else 0)
                xv = wk.tile([P, E], S_DT, tag="xv")
                nc.vector.tensor_tensor(xv[:, :], xe[:, :], vf[:, :], AluOpType.mult)

                # ---- horizontal 3-tap sums (free-dim shifts, then overwrite
                # the w=0 / w=255 columns with clipped 2-term sums)
                hs = wk.tile([P, E], S_DT, tag="hs")
                nc.vector.tensor_tensor(
                    hs[:, 1 : E - 1], xv[:, 0 : E - 2], xv[:, 1 : E - 1], AluOpType.add
                )
                nc.vector.tensor_tensor(
                    hs[:, 1 : E - 1], hs[:, 1 : E - 1], xv[:, 2:E], AluOpType.add
                )
                cs = wk.tile([P, E], BF16, tag="cs")
                nc.vector.tensor_tensor(
                    cs[:, 1 : E - 1], vf[:, 0 : E - 2], vf[:, 1 : E - 1], AluOpType.add
                )
                nc.vector.tensor_tensor(
                    cs[:, 1 : E - 1], cs[:, 1 : E - 1], vf[:, 2:E], AluOpType.add
                )
                hs3 = hs[:, :].rearrange("p (r w) -> p r w", w=W)
                xv3 = xv[:, :].rearrange("p (r w) -> p r w", w=W)
                cs3 = cs[:, :].rearrange("p (r w) -> p r w", w=W)
                vf3 = vf[:, :].rearrange("p (r w) -> p r w", w=W)
                nc.vector.tensor_tensor(
                    hs3[:, :, 0:1], xv3[:, :, 0:1], xv3[:, :, 1:2], AluOpType.add
                )
                nc.vector.tensor_tensor(
                    hs3[:, :, W - 1 : W],
                    xv3[:, :, W - 2 : W - 1],
                    xv3[:, :, W - 1 : W],
                    AluOpType.add,
                )
                nc.vector.tensor_tensor(
                    cs3[:, :, 0:1], vf3[:, :, 0:1], vf3[:, :, 1:2], AluOpType.add
                )
                nc.vector.tensor_tensor(
                    cs3[:, :, W - 1 : W],
                    vf3[:, :, W - 2 : W - 1],
                    vf3[:, :, W - 1 : W],
                    AluOpType.add,
                )

                # ---- vertical 3-tap sums on interior rows
                vs = iop.tile([P, KW], S_DT, tag="vs")
                nc.vector.tensor_tensor(
                    vs[:, :], hs[:, 0:KW], hs[:, W : (K + 1) * W], AluOpType.add
                )
                nc.vector.tensor_tensor(
                    vs[:, :], vs[:, :], hs[:, 2 * W : E], AluOpType.add
                )
                csum = wk.tile([P, KW], BF16, tag="csum")
                nc.vector.tensor_tensor(
                    csum[:, :], cs[:, 0:KW], cs[:, W : (K + 1) * W], AluOpType.add
                )
                nc.vector.tensor_tensor(
                    csum[:, :], csum[:, :], cs[:, 2 * W : E], AluOpType.add
                )
                # ---- 1/cc on ScalarE as Exp(-Ln(cc)); counts are exact
                # small ints so both splines are well-conditioned
                cc = iop.tile([P, KW], F32, tag="cc")
                nc.scalar.activation(
                    cc[:, :], csum[:, :], mybir.ActivationFunctionType.Ln
                )
                nc.scalar.activation(
                    cc[:, :],
                    cc[:, :],
                    mybir.ActivationFunctionType.Exp,
                    scale=-1.0,
                )
                ot = vs if not S_BF16 else iop.tile([P, KW], F32, tag="ot")
                nc.vector.tensor_tensor(ot[:, :], vs[:, :], cc[:, :], AluOpType.mult)

                # ---- final blend: faulty (|x|>2.5) -> mean, else passthrough
                if _SELECT_BAND is not None:
                    nc.vector._custom_dve(
                        _SELECT_BAND,
                        out=ot[:, :],
                        in0=ot[:, :],
                        in1=xe[:, W : (K + 1) * W],
                        s0=2.5,
                        s1=-2.5,
                    )
                else:
                    kp = wk.tile([P, KW], mybir.dt.uint8, tag="kp")
                    if BITCAST_MASK:
                        nc.vector.tensor_scalar(
                            kp[:, :],
                            xe[:, W : (K + 1) * W].bitcast(I32),
                            SIGN_CLEAR,
                            ABS_BITS_25,
                            AluOpType.bitwise_and,
                            AluOpType.is_le,
                        )
                    else:
                        nc.vector.tensor_scalar(
                            kp[:, :],
                            xe[:, W : (K + 1) * W],
                            2.5,
                            None,
                            AluOpType.is_le,
                        )
                    nc.vector.copy_predicated(
                        ot[:, :], kp[:, :], xe[:, W : (K + 1) * W]
                    )

                nc.sync.dma_start(out[:, t * KW : (t + 1) * KW], ot[:, :])

    nc.compile()
    return nc


def _get_nc():
    global _NC_CACHE
    if _NC_CACHE is None:
        _NC_CACHE = build_nc()
    return _NC_CACHE


def kernel(x: np.ndarray) -> np.ndarray:
    assert x.shape == (N, C, H, W) and x.dtype == np.float32
    xs = np.ascontiguousarray(x).reshape(NCORES, P, FREE)
    in_maps = [{"x": xs[i]} for i in range(NCORES)]
    res = bass_utils.run_bass_kernel_spmd(
        _get_nc(), in_maps, core_ids=list(range(NCORES))
    )
    out = np.stack([res.results[i]["out"] for i in range(NCORES)])
    return out.reshape(N, C, H, W)


# revision 17
# speedup vs baseline: 1.0249x; 1.0249x over previous
"""3x3 windowed mean-imputation (nn_Averager) on 8 trn2 NeuronCores.

out = where(|x| > 2.5, Wsum3x3(x*valid) / Wcnt3x3(valid), x)
valid = |x| < 2.5, SAME zero padding. (Wcnt >= 1 at every faulty point for
this input; |x| == 2.5 never occurs — both verified offline in test.py.)

Sharding: pure data parallel. x is (16, 64, 256, 256) fp32; each of the 8
cores gets 2 N slices = 128 images, laid out as [128 partitions, 65536 free]
(partition = image, free = flattened h*256+w). Both window axes are free-dim
shifts; vertical pass uses 1-row halos, horizontal edge columns are
overwritten with 2-term sums. Counts ride in bf16 (exact for ints <= 9).
"""

import sys

sys.path.insert(0, "/opt/trn_rl_repo")

import numpy as np

import concourse.bacc as bacc
import concourse.mybir as mybir
from concourse import bass_utils
from concourse.mybir import AluOpType
from concourse.tile import TileContext

N, C, H, W = 16, 64, 256, 256
NCORES = 8
P = (N // NCORES) * C  # 128 images per core = 128 partitions
FREE = H * W  # 65536
K = 16  # image rows per tile
R = K + 2  # with halo rows
E = R * W  # extended tile free size
KW = K * W
BIG = 1.0e9  # halo fill: |BIG| > 2.5 so vf=0 and x*vf=0

F32 = mybir.dt.float32
BF16 = mybir.dt.bfloat16
I32 = mybir.dt.int32

# s-path (windowed value sums) dtype: BF16 halves the shifted-add cost (2x
# DVE mode) at ~5e-3 rel err on imputed points; F32 keeps absmax err ~4e-6.
S_BF16 = False
S_DT = BF16 if S_BF16 else F32

# masks via int32 bitcast (|x| as sign-bit clear, compare in int domain);
# falls back to an |x| pass on ScalarE if disabled.
BITCAST_MASK = False
ABS_BITS_25 = 0x40200000  # bits(2.5f)
SIGN_CLEAR = 0x7FFFFFFF

_NC_CACHE = None


def _register_select_band():
    """Custom DVE op: out = select(in1 > s0 or in1 < s1, in0, in1).

    One 1x-rate Vector op replacing mask-gen + copy_predicated for the final
    blend (in0 = window mean, in1 = x)."""
    from concourse import dve_ops
    from concourse.dve_spec import C0, C1, Spec, Src0, Src1, _has_src1
    from concourse.dve_spec import lower as dve_lower
    from concourse.dve_spec import select as dve_select
    from concourse.dve_uop import DveOpSpec

    name = "SELECT_BAND_ANT"
    if name in dve_ops._SUB_OPCODE_FOR_NAME:
        return next(op for op in dve_ops.OPS if op.name == name)

    spec = Spec(
        body=dve_select((Src1 > C0) | (Src1 < C1), Src0, Src1),
        reference=lambda in0, in1, s0, s1, imm2: np.where(
            (in1 > s0) | (in1 < s1), in0, in1
        ).astype(np.float32),
    )
    row = max(dve_ops._SUB_OPCODE_FOR_NAME.values()) + 1
    assert row < 0x20
    dve_ops._SUB_OPCODE_FOR_NAME[name] = row
    try:
        shas = {}
        for ver in ("v3", "v4"):
            tmp = DveOpSpec(
                name=name,
                opcode=row,
                uops=dve_lower(spec, ver=ver),
                rd1_en=_has_src1(spec),
            )
            shas[ver] = tmp.sha(ver)
        op = dve_ops.DveOp(name, spec, subdim=False, uops_sha=shas)
    except Exception:
        del dve_ops._SUB_OPCODE_FOR_NAME[name]
        raise
    dve_ops.OPS.append(op)
    dve_ops.CUSTOM_DVE_SPECS[name] = spec
    return op


try:
    _SELECT_BAND = _register_select_band()
except Exception:
    _SELECT_BAND = None


def build_nc():
    nc = bacc.Bacc("TRN2", target_bir_lowering=False)
    # bias constant for the Relu(2.5 - |x|) activation
    _c = nc.alloc_sbuf_tensor("const-float32-2.5", [128, 1], F32)
    nc.gpsimd.memset(_c.ap(), 2.5)
    nc.const_aps.aps[(F32, 2.5)] = _c.ap()
    nc.all_engine_barrier()
    x = nc.dram_tensor("x", [P, FREE], F32, kind="ExternalInput")
    out = nc.dram_tensor("out", [P, FREE], F32, kind="ExternalOutput")

    with TileContext(nc) as tc:
        with (
            tc.tile_pool(name="io", bufs=2) as iop,
            tc.tile_pool(name="wk", bufs=1) as wk,
        ):
            n_tiles = H // K
            for t in range(n_tiles):
                xe = iop.tile([P, E], F32, tag="xe")
                # ---- load x rows [t*K-1, t*K+K+1) with BIG-filled halo at
                # image top/bottom (BIG -> invalid, contributes 0 to sums)
                if t == 0:
                    nc.vector.memset(xe[:, 0:W], BIG)
                    nc.sync.dma_start(xe[:, W:E], x[:, 0 : (K + 1) * W])
                elif t == n_tiles - 1:
                    nc.sync.dma_start(
                        xe[:, 0 : (K + 1) * W], x[:, (t * K - 1) * W : FREE]
                    )
                    nc.vector.memset(xe[:, (K + 1) * W : E], BIG)
                else:
                    nc.sync.dma_start(
                        xe[:, :], x[:, (t * K - 1) * W : (t * K + K + 1) * W]
                    )

                # ---- vf = (|x| < 2.5) as bf16 0/1
                vf = iop.tile([P, E], BF16, tag="vf")
                if BITCAST_MASK:
                    nc.vector.tensor_scalar(
                        vf[:, :],
                        xe[:, :].bitcast(I32),
                        SIGN_CLEAR,
                        ABS_BITS_25,
                        AluOpType.bitwise_and,
                        AluOpType.is_lt,
                    )
                else:
                    ab = wk.tile([P, E], F32, tag="ab")
                    nc.scalar.activation(
                        ab[:, :], xe[:, :], mybir.ActivationFunctionType.Abs
                    )
                    nc.scalar.activation(
                        ab[:, :],
                        ab[:, :],
                        mybir.ActivationFunctionType.Relu,
                        bias=2.5,
                        scale=-1.0,
                    )
                    nc.scalar.activation(
                        vf[:, :],
                        ab[:, :],
                        mybir.ActivationFunctionType.Tanh,
                        scale=1.0e38,
                    )
                # xv = x * vf  (valid values, else 0)
                xv = wk.tile([P, E], S_DT, tag="xv")
                nc.vector.tensor_tensor(xv[:, :], xe[:, :], vf[:, :], AluOpType.mult)

                # ---- horizontal 3-tap sums (free-dim shifts, then overwrite
                # the w=0 / w=255 columns with clipped 2-term sums)
                hs = wk.tile([P, E], S_DT, tag="hs")
                nc.vector.tensor_tensor(
                    hs[:, 1 : E - 1], xv[:, 0 : E - 2], xv[:, 1 : E - 1], AluOpType.add
                )
                nc.vector.tensor_tensor(
                    hs[:, 1 : E - 1], hs[:, 1 : E - 1], xv[:, 2:E], AluOpType.add
                )
                cs = wk.tile([P, E], BF16, tag="cs")
                nc.vector.tensor_tensor(
                    cs[:, 1 : E - 1], vf[:, 0 : E - 2], vf[:, 1 : E - 1], AluOpType.add
                )
                nc.vector.tensor_tensor(
                    cs[:, 1 : E - 1], cs[:, 1 : E - 1], vf[:, 2:E], AluOpType.add
                )
                hs3 = hs[:, :].rearrange("p (r w) -> p r w", w=W)
                xv3 = xv[:, :].rearrange("p (r w) -> p r w", w=W)
                cs3 = cs[:, :].rearrange("p (r w) -> p r w", w=W)
                vf3 = vf[:, :].rearrange("p (r w) -> p r w", w=W)
                nc.vector.tensor_tensor(
                    hs3[:, :, 0:1], xv3[:, :, 0:1], xv3[:, :, 1:2], AluOpType.add
                )
                nc.vector.tensor_tensor(
                    hs3[:, :, W - 1 : W],
                    xv3[:, :, W - 2 : W - 1],
                    xv3[:, :, W - 1 : W],
                    AluOpType.add,
                )
                nc.vector.tensor_tensor(
                    cs3[:, :, 0:1], vf3[:, :, 0:1], vf3[:, :, 1:2], AluOpType.add
                )
                nc.vector.tensor_tensor(
                    cs3[:, :, W - 1 : W],
                    vf3[:, :, W - 2 : W - 1],
                    vf3[:, :, W - 1 : W],
                    AluOpType.add,
                )

                # ---- vertical 3-tap sums on interior rows
                vs = iop.tile([P, KW], S_DT, tag="vs")
                nc.vector.tensor_tensor(
                    vs[:, :], hs[:, 0:KW], hs[:, W : (K + 1) * W], AluOpType.add
                )
                nc.vector.tensor_tensor(
                    vs[:, :], vs[:, :], hs[:, 2 * W : E], AluOpType.add
                )
                csum = wk.tile([P, KW], BF16, tag="csum")
                nc.vector.tensor_tensor(
                    csum[:, :], cs[:, 0:KW], cs[:, W : (K + 1) * W], AluOpType.add
                )
                nc.vector.tensor_tensor(
                    csum[:, :], csum[:, :], cs[:, 2 * W : E], AluOpType.add
                )
                # exact int counts bf16 -> fp32 on ScalarE (recip needs f32
                # bits); then 1/cc in place on the DVE
                cc = iop.tile([P, KW], F32, tag="cc")
                nc.scalar.copy(cc[:, :], csum[:, :])
                nc.vector.reciprocal_approx_fast(cc[:, :], cc[:, :])
                ot = vs if not S_BF16 else iop.tile([P, KW], F32, tag="ot")
                nc.vector.tensor_tensor(ot[:, :], vs[:, :], cc[:, :], AluOpType.mult)

                # ---- final blend: faulty (|x|>2.5) -> mean, else passthrough
                if _SELECT_BAND is not None:
                    nc.vector._custom_dve(
                        _SELECT_BAND,
                        out=ot[:, :],
                        in0=ot[:, :],
                        in1=xe[:, W : (K + 1) * W],
                        s0=2.5,
                        s1=-2.5,
                    )
                else:
                    kp = wk.tile([P, KW], mybir.dt.uint8, tag="kp")
                    if BITCAST_MASK:
                        nc.vector.tensor_scalar(
                            kp[:, :],
                            xe[:, W : (K + 1) * W].bitcast(I32),
                            SIGN_CLEAR,
                            ABS_BITS_25,
                            AluOpType.bitwise_and,
                            AluOpType.is_le,
                        )
                    else:
                        nc.vector.tensor_scalar(
                            kp[:, :],
                            xe[:, W : (K + 1) * W],
                            2.5,
                            None,
                            AluOpType.is_le,
                        )
                    nc.vector.copy_predicated(
                        ot[:, :], kp[:, :], xe[:, W : (K + 1) * W]
                    )

                nc.sync.dma_start(out[:, t * KW : (t + 1) * KW], ot[:, :])

    nc.compile()
    return nc


def _get_nc():
    global _NC_CACHE
    if _NC_CACHE is None:
        _NC_CACHE = build_nc()
    return _NC_CACHE


def kernel(x: np.ndarray) -> np.ndarray:
    assert x.shape == (N, C, H, W) and x.dtype == np.float32
    xs = np.ascontiguousarray(x).reshape(NCORES, P, FREE)
    in_maps = [{"x": xs[i]} for i in range(NCORES)]
    res = bass_utils.run_bass_kernel_spmd(
        _get_nc(), in_maps, core_ids=list(range(NCORES))
    )
    out = np.stack([res.results[i]["out"] for i in range(NCORES)])
    return out.reshape(N, C, H, W)


# revision 18
# speedup vs baseline: 1.1003x; 1.0736x over previous
"""3x3 windowed mean-imputation (nn_Averager) on 8 trn2 NeuronCores.

out = where(|x| > 2.5, Wsum3x3(x*valid) / Wcnt3x3(valid), x)
valid = |x| < 2.5, SAME zero padding. (Wcnt >= 1 at every faulty point for
this input; |x| == 2.5 never occurs — both verified offline in test.py.)

Sharding: pure data parallel. x is (16, 64, 256, 256) fp32; each of the 8
cores gets 2 N slices = 128 images, laid out as [128 partitions, 65536 free]
(partition = image, free = flattened h*256+w). Both window axes are free-dim
shifts; vertical pass uses 1-row halos, horizontal edge columns are
overwritten with 2-term sums. Counts ride in bf16 (exact for ints <= 9).
"""

import sys

sys.path.insert(0, "/opt/trn_rl_repo")

import numpy as np

import concourse.bacc as bacc
import concourse.mybir as mybir
from concourse import bass_utils
from concourse.mybir import AluOpType
from concourse.tile import TileContext

N, C, H, W = 16, 64, 256, 256
NCORES = 8
P = (N // NCORES) * C  # 128 images per core = 128 partitions
FREE = H * W  # 65536
K = 16  # image rows per tile
R = K + 2  # with halo rows
E = R * W  # extended tile free size
KW = K * W
BIG = 1.0e9  # halo fill: |BIG| > 2.5 so vf=0 and x*vf=0

F32 = mybir.dt.float32
BF16 = mybir.dt.bfloat16
I32 = mybir.dt.int32

# s-path (windowed value sums) dtype: BF16 halves the shifted-add cost (2x
# DVE mode) at ~5e-3 rel err on imputed points; F32 keeps absmax err ~4e-6.
S_BF16 = False
S_DT = BF16 if S_BF16 else F32

# masks via int32 bitcast (|x| as sign-bit clear, compare in int domain);
# falls back to an |x| pass on ScalarE if disabled.
BITCAST_MASK = False
ABS_BITS_25 = 0x40200000  # bits(2.5f)
SIGN_CLEAR = 0x7FFFFFFF

_NC_CACHE = None


def _register_select_band():
    """Custom DVE op: out = select(in1 > s0 or in1 < s1, in0, in1).

    One 1x-rate Vector op replacing mask-gen + copy_predicated for the final
    blend (in0 = window mean, in1 = x)."""
    from concourse import dve_ops
    from concourse.dve_spec import C0, C1, Spec, Src0, Src1, _has_src1
    from concourse.dve_spec import lower as dve_lower
    from concourse.dve_spec import select as dve_select
    from concourse.dve_uop import DveOpSpec

    name = "SELECT_BAND_ANT"
    if name in dve_ops._SUB_OPCODE_FOR_NAME:
        return next(op for op in dve_ops.OPS if op.name == name)

    spec = Spec(
        body=dve_select((Src1 > C0) | (Src1 < C1), Src0, Src1),
        reference=lambda in0, in1, s0, s1, imm2: np.where(
            (in1 > s0) | (in1 < s1), in0, in1
        ).astype(np.float32),
    )
    row = max(dve_ops._SUB_OPCODE_FOR_NAME.values()) + 1
    assert row < 0x20
    dve_ops._SUB_OPCODE_FOR_NAME[name] = row
    try:
        shas = {}
        for ver in ("v3", "v4"):
            tmp = DveOpSpec(
                name=name,
                opcode=row,
                uops=dve_lower(spec, ver=ver),
                rd1_en=_has_src1(spec),
            )
            shas[ver] = tmp.sha(ver)
        op = dve_ops.DveOp(name, spec, subdim=False, uops_sha=shas)
    except Exception:
        del dve_ops._SUB_OPCODE_FOR_NAME[name]
        raise
    dve_ops.OPS.append(op)
    dve_ops.CUSTOM_DVE_SPECS[name] = spec
    return op


try:
    _SELECT_BAND = _register_select_band()
except Exception:
    _SELECT_BAND = None


def build_nc():
    nc = bacc.Bacc("TRN2", target_bir_lowering=False)
    # bias constant for the Relu(2.5 - |x|) activation
    _c = nc.alloc_sbuf_tensor("const-float32-2.5", [128, 1], F32)
    nc.gpsimd.memset(_c.ap(), 2.5)
    nc.const_aps.aps[(F32, 2.5)] = _c.ap()
    nc.all_engine_barrier()
    x = nc.dram_tensor("x", [P, FREE], F32, kind="ExternalInput")
    out = nc.dram_tensor("out", [P, FREE], F32, kind="ExternalOutput")

    with TileContext(nc) as tc:
        with (
            tc.tile_pool(name="io", bufs=2) as iop,
            tc.tile_pool(name="wk", bufs=1) as wk,
        ):
            n_tiles = H // K
            for t in range(n_tiles):
                xe = iop.tile([P, E], F32, tag="xe")
                # ---- load x rows [t*K-1, t*K+K+1) with BIG-filled halo at
                # image top/bottom (BIG -> invalid, contributes 0 to sums)
                if t == 0:
                    nc.vector.memset(xe[:, 0:W], BIG)
                    nc.sync.dma_start(xe[:, W:E], x[:, 0 : (K + 1) * W])
                elif t == n_tiles - 1:
                    nc.sync.dma_start(
                        xe[:, 0 : (K + 1) * W], x[:, (t * K - 1) * W : FREE]
                    )
                    nc.vector.memset(xe[:, (K + 1) * W : E], BIG)
                else:
                    nc.sync.dma_start(
                        xe[:, :], x[:, (t * K - 1) * W : (t * K + K + 1) * W]
                    )

                # ---- vf = (|x| < 2.5) as bf16 0/1
                vf = iop.tile([P, E], BF16, tag="vf")
                if BITCAST_MASK:
                    nc.vector.tensor_scalar(
                        vf[:, :],
                        xe[:, :].bitcast(I32),
                        SIGN_CLEAR,
                        ABS_BITS_25,
                        AluOpType.bitwise_and,
                        AluOpType.is_lt,
                    )
                else:
                    ab = wk.tile([P, E], F32, tag="ab")
                    nc.scalar.activation(
                        ab[:, :], xe[:, :], mybir.ActivationFunctionType.Abs
                    )
                    nc.scalar.activation(
                        ab[:, :],
                        ab[:, :],
                        mybir.ActivationFunctionType.Relu,
                        bias=2.5,
                        scale=-1.0,
                    )
                    nc.scalar.activation(
                        vf[:, :],
                        ab[:, :],
                        mybir.ActivationFunctionType.Tanh,
                        scale=1.0e38,
                    )
                # ---- count path FIRST so the ScalarE bf16->f32 convert
                # overlaps the s-path adds below (keeps the DVE gapless)
                cs = wk.tile([P, E], BF16, tag="cs")
                nc.vector.tensor_tensor(
                    cs[:, 1 : E - 1], vf[:, 0 : E - 2], vf[:, 1 : E - 1], AluOpType.add
                )
                nc.vector.tensor_tensor(
                    cs[:, 1 : E - 1], cs[:, 1 : E - 1], vf[:, 2:E], AluOpType.add
                )
                cs3 = cs[:, :].rearrange("p (r w) -> p r w", w=W)
                vf3 = vf[:, :].rearrange("p (r w) -> p r w", w=W)
                nc.vector.tensor_tensor(
                    cs3[:, :, 0:1], vf3[:, :, 0:1], vf3[:, :, 1:2], AluOpType.add
                )
                nc.vector.tensor_tensor(
                    cs3[:, :, W - 1 : W],
                    vf3[:, :, W - 2 : W - 1],
                    vf3[:, :, W - 1 : W],
                    AluOpType.add,
                )
                csum = wk.tile([P, KW], BF16, tag="csum")
                nc.vector.tensor_tensor(
                    csum[:, :], cs[:, 0:KW], cs[:, W : (K + 1) * W], AluOpType.add
                )
                nc.vector.tensor_tensor(
                    csum[:, :], csum[:, :], cs[:, 2 * W : E], AluOpType.add
                )
                # exact int counts bf16 -> fp32 on ScalarE (recip needs f32)
                cc = iop.tile([P, KW], F32, tag="cc")
                nc.scalar.copy(cc[:, :], csum[:, :])

                # ---- s path: xv = x * vf, then horizontal 3-tap sums
                # (free-dim shifts; w=0 / w=255 columns overwritten with
                # clipped 2-term sums)
                xv = wk.tile([P, E], S_DT, tag="xv")
                nc.vector.tensor_tensor(xv[:, :], xe[:, :], vf[:, :], AluOpType.mult)
                hs = wk.tile([P, E], S_DT, tag="hs")
                nc.vector.tensor_tensor(
                    hs[:, 1 : E - 1], xv[:, 0 : E - 2], xv[:, 1 : E - 1], AluOpType.add
                )
                nc.vector.tensor_tensor(
                    hs[:, 1 : E - 1], hs[:, 1 : E - 1], xv[:, 2:E], AluOpType.add
                )
                hs3 = hs[:, :].rearrange("p (r w) -> p r w", w=W)
                xv3 = xv[:, :].rearrange("p (r w) -> p r w", w=W)
                nc.vector.tensor_tensor(
                    hs3[:, :, 0:1], xv3[:, :, 0:1], xv3[:, :, 1:2], AluOpType.add
                )
                nc.vector.tensor_tensor(
                    hs3[:, :, W - 1 : W],
                    xv3[:, :, W - 2 : W - 1],
                    xv3[:, :, W - 1 : W],
                    AluOpType.add,
                )

                # ---- vertical 3-tap sums on interior rows
                vs = iop.tile([P, KW], S_DT, tag="vs")
                nc.vector.tensor_tensor(
                    vs[:, :], hs[:, 0:KW], hs[:, W : (K + 1) * W], AluOpType.add
                )
                nc.vector.tensor_tensor(
                    vs[:, :], vs[:, :], hs[:, 2 * W : E], AluOpType.add
                )
                # 1/cc in place on the DVE
                nc.vector.reciprocal_approx_fast(cc[:, :], cc[:, :])
                ot = vs if not S_BF16 else iop.tile([P, KW], F32, tag="ot")
                nc.vector.tensor_tensor(ot[:, :], vs[:, :], cc[:, :], AluOpType.mult)

                # ---- final blend: faulty (|x|>2.5) -> mean, else passthrough
                if _SELECT_BAND is not None:
                    nc.vector._custom_dve(
                        _SELECT_BAND,
                        out=ot[:, :],
                        in0=ot[:, :],
                        in1=xe[:, W : (K + 1) * W],
                        s0=2.5,
                        s1=-2.5,
                    )
                else:
                    kp = wk.tile([P, KW], mybir.dt.uint8, tag="kp")
                    if BITCAST_MASK:
                        nc.vector.tensor_scalar(
                            kp[:, :],
                            xe[:, W : (K + 1) * W].bitcast(I32),
                            SIGN_CLEAR,
                            ABS_BITS_25,
                            AluOpType.bitwise_and,
                            AluOpType.is_le,
                        )
                    else:
                        nc.vector.tensor_scalar(
                            kp[:, :],
                            xe[:, W : (K + 1) * W],
                            2.5,
                            None,
                            AluOpType.is_le,
                        )
                    nc.vector.copy_predicated(
                        ot[:, :], kp[:, :], xe[:, W : (K + 1) * W]
                    )

                nc.sync.dma_start(out[:, t * KW : (t + 1) * KW], ot[:, :])

    nc.compile()
    return nc


def _get_nc():
    global _NC_CACHE
    if _NC_CACHE is None:
        _NC_CACHE = build_nc()
    return _NC_CACHE


def kernel(x: np.ndarray) -> np.ndarray:
    assert x.shape == (N, C, H, W) and x.dtype == np.float32
    xs = np.ascontiguousarray(x).reshape(NCORES, P, FREE)
    in_maps = [{"x": xs[i]} for i in range(NCORES)]
    res = bass_utils.run_bass_kernel_spmd(
        _get_nc(), in_maps, core_ids=list(range(NCORES))
    )
    out = np.stack([res.results[i]["out"] for i in range(NCORES)])
    return out.reshape(N, C, H, W)


# revision 23
# speedup vs baseline: 1.5166x; 1.3783x over previous
"""3x3 windowed mean-imputation (nn_Averager) on 8 trn2 NeuronCores.

out = where(|x| > 2.5, Wsum3x3(x*valid) / Wcnt3x3(valid), x)
valid = |x| < 2.5, SAME zero padding. (Wcnt >= 1 at every faulty point for
this input; |x| == 2.5 never occurs — both verified offline in test.py.)

Sharding: pure data parallel. x is (16, 64, 256, 256) fp32; each of the 8
cores gets 2 N slices = 128 images, laid out as [128 partitions, 65536 free]
(partition = image, free = flattened h*256+w). Both window axes are free-dim
shifts; vertical pass uses 1-row halos, horizontal edge columns are
overwritten with 2-term sums. Counts ride in bf16 (exact for ints <= 9).
"""

import sys

sys.path.insert(0, "/opt/trn_rl_repo")

import numpy as np

import concourse.bacc as bacc
import concourse.mybir as mybir
from concourse import bass_utils
from concourse.mybir import AluOpType
from concourse.tile import TileContext

N, C, H, W = 16, 64, 256, 256
NCORES = 8
P = (N // NCORES) * C  # 128 images per core = 128 partitions
FREE = H * W  # 65536
K = 16  # image rows per tile
R = K + 2  # with halo rows
E = R * W  # extended tile free size
KW = K * W
BIG = 1.0e9  # halo fill: |BIG| > 2.5 so vf=0 and x*vf=0

F32 = mybir.dt.float32
BF16 = mybir.dt.bfloat16
I32 = mybir.dt.int32

# s-path (windowed value sums) dtype: BF16 halves the shifted-add cost (2x
# DVE mode) at ~5e-3 rel err on imputed points; F32 keeps absmax err ~4e-6.
S_BF16 = False
S_DT = BF16 if S_BF16 else F32

# masks via int32 bitcast (|x| as sign-bit clear, compare in int domain);
# falls back to an |x| pass on ScalarE if disabled.
BITCAST_MASK = False
ABS_BITS_25 = 0x40200000  # bits(2.5f)
SIGN_CLEAR = 0x7FFFFFFF

_NC_CACHE = None

# "A": partition = image, all sums on the DVE (no PE).
# "B": partition = h-row; vertical 3-tap sums run as band-matrix matmuls on
#      the otherwise-idle TensorEngine, horizontal sums stay on the DVE.
LAYOUT = "B"


def _register_select_band():
    """Custom DVE op: out = select(in1 > s0 or in1 < s1, in0, in1).

    One 1x-rate Vector op replacing mask-gen + copy_predicated for the final
    blend (in0 = window mean, in1 = x)."""
    from concourse import dve_ops
    from concourse.dve_spec import C0, C1, Spec, Src0, Src1, _has_src1
    from concourse.dve_spec import lower as dve_lower
    from concourse.dve_spec import select as dve_select
    from concourse.dve_uop import DveOpSpec

    name = "SELECT_BAND_ANT"
    if name in dve_ops._SUB_OPCODE_FOR_NAME:
        return next(op for op in dve_ops.OPS if op.name == name)

    spec = Spec(
        body=dve_select((Src1 > C0) | (Src1 < C1), Src0, Src1),
        reference=lambda in0, in1, s0, s1, imm2: np.where(
            (in1 > s0) | (in1 < s1), in0, in1
        ).astype(np.float32),
    )
    row = max(dve_ops._SUB_OPCODE_FOR_NAME.values()) + 1
    assert row < 0x20
    dve_ops._SUB_OPCODE_FOR_NAME[name] = row
    try:
        shas = {}
        for ver in ("v3", "v4"):
            tmp = DveOpSpec(
                name=name,
                opcode=row,
                uops=dve_lower(spec, ver=ver),
                rd1_en=_has_src1(spec),
            )
            shas[ver] = tmp.sha(ver)
        op = dve_ops.DveOp(name, spec, subdim=False, uops_sha=shas)
    except Exception:
        del dve_ops._SUB_OPCODE_FOR_NAME[name]
        raise
    dve_ops.OPS.append(op)
    dve_ops.CUSTOM_DVE_SPECS[name] = spec
    return op


try:
    _SELECT_BAND = _register_select_band()
except Exception:
    _SELECT_BAND = None


def build_nc():
    nc = bacc.Bacc("TRN2", target_bir_lowering=False)
    # bias constant for the Relu(2.5 - |x|) activation
    _c = nc.alloc_sbuf_tensor("const-float32-2.5", [128, 1], F32)
    nc.gpsimd.memset(_c.ap(), 2.5)
    nc.const_aps.aps[(F32, 2.5)] = _c.ap()
    nc.all_engine_barrier()
    x = nc.dram_tensor("x", [P, FREE], F32, kind="ExternalInput")
    out = nc.dram_tensor("out", [P, FREE], F32, kind="ExternalOutput")

    with TileContext(nc) as tc:
        with (
            tc.tile_pool(name="io", bufs=2) as iop,
            tc.tile_pool(name="wk", bufs=1) as wk,
        ):
            n_tiles = H // K
            for t in range(n_tiles):
                xe = iop.tile([P, E], F32, tag="xe")
                # ---- load x rows [t*K-1, t*K+K+1) with BIG-filled halo at
                # image top/bottom (BIG -> invalid, contributes 0 to sums)
                if t == 0:
                    nc.vector.memset(xe[:, 0:W], BIG)
                    nc.sync.dma_start(xe[:, W:E], x[:, 0 : (K + 1) * W])
                elif t == n_tiles - 1:
                    nc.sync.dma_start(
                        xe[:, 0 : (K + 1) * W], x[:, (t * K - 1) * W : FREE]
                    )
                    nc.vector.memset(xe[:, (K + 1) * W : E], BIG)
                else:
                    nc.sync.dma_start(
                        xe[:, :], x[:, (t * K - 1) * W : (t * K + K + 1) * W]
                    )

                # ---- vf = (|x| < 2.5) as bf16 0/1
                vf = iop.tile([P, E], BF16, tag="vf")
                if BITCAST_MASK:
                    nc.vector.tensor_scalar(
                        vf[:, :],
                        xe[:, :].bitcast(I32),
                        SIGN_CLEAR,
                        ABS_BITS_25,
                        AluOpType.bitwise_and,
                        AluOpType.is_lt,
                    )
                else:
                    ab = wk.tile([P, E], F32, tag="ab")
                    nc.scalar.activation(
                        ab[:, :], xe[:, :], mybir.ActivationFunctionType.Abs
                    )
                    nc.scalar.activation(
                        ab[:, :],
                        ab[:, :],
                        mybir.ActivationFunctionType.Relu,
                        bias=2.5,
                        scale=-1.0,
                    )
                    nc.scalar.activation(
                        vf[:, :],
                        ab[:, :],
                        mybir.ActivationFunctionType.Tanh,
                        scale=1.0e38,
                    )
                # ---- count path FIRST so the ScalarE bf16->f32 convert
                # overlaps the s-path adds below (keeps the DVE gapless)
                cs = wk.tile([P, E], BF16, tag="cs")
                nc.vector.tensor_tensor(
                    cs[:, 1 : E - 1], vf[:, 0 : E - 2], vf[:, 1 : E - 1], AluOpType.add
                )
                nc.vector.tensor_tensor(
                    cs[:, 1 : E - 1], cs[:, 1 : E - 1], vf[:, 2:E], AluOpType.add
                )
                cs3 = cs[:, :].rearrange("p (r w) -> p r w", w=W)
                vf3 = vf[:, :].rearrange("p (r w) -> p r w", w=W)
                nc.vector.tensor_tensor(
                    cs3[:, :, 0:1], vf3[:, :, 0:1], vf3[:, :, 1:2], AluOpType.add
                )
                nc.vector.tensor_tensor(
                    cs3[:, :, W - 1 : W],
                    vf3[:, :, W - 2 : W - 1],
                    vf3[:, :, W - 1 : W],
                    AluOpType.add,
                )
                csum = wk.tile([P, KW], BF16, tag="csum")
                nc.vector.tensor_tensor(
                    csum[:, :], cs[:, 0:KW], cs[:, W : (K + 1) * W], AluOpType.add
                )
                nc.vector.tensor_tensor(
                    csum[:, :], csum[:, :], cs[:, 2 * W : E], AluOpType.add
                )
                # exact int counts bf16 -> fp32 on ScalarE (recip needs f32)
                cc = iop.tile([P, KW], F32, tag="cc")
                nc.scalar.copy(cc[:, :], csum[:, :])

                # ---- s path: xv = x * vf, then horizontal 3-tap sums
                # (free-dim shifts; w=0 / w=255 columns overwritten with
                # clipped 2-term sums)
                xv = wk.tile([P, E], S_DT, tag="xv")
                nc.vector.tensor_tensor(xv[:, :], xe[:, :], vf[:, :], AluOpType.mult)
                hs = wk.tile([P, E], S_DT, tag="hs")
                nc.vector.tensor_tensor(
                    hs[:, 1 : E - 1], xv[:, 0 : E - 2], xv[:, 1 : E - 1], AluOpType.add
                )
                nc.vector.tensor_tensor(
                    hs[:, 1 : E - 1], hs[:, 1 : E - 1], xv[:, 2:E], AluOpType.add
                )
                hs3 = hs[:, :].rearrange("p (r w) -> p r w", w=W)
                xv3 = xv[:, :].rearrange("p (r w) -> p r w", w=W)
                nc.vector.tensor_tensor(
                    hs3[:, :, 0:1], xv3[:, :, 0:1], xv3[:, :, 1:2], AluOpType.add
                )
                nc.vector.tensor_tensor(
                    hs3[:, :, W - 1 : W],
                    xv3[:, :, W - 2 : W - 1],
                    xv3[:, :, W - 1 : W],
                    AluOpType.add,
                )

                # ---- vertical 3-tap sums on interior rows
                vs = iop.tile([P, KW], S_DT, tag="vs")
                nc.vector.tensor_tensor(
                    vs[:, :], hs[:, 0:KW], hs[:, W : (K + 1) * W], AluOpType.add
                )
                nc.vector.tensor_tensor(
                    vs[:, :], vs[:, :], hs[:, 2 * W : E], AluOpType.add
                )
                # 1/cc in place on the DVE
                nc.vector.reciprocal_approx_fast(cc[:, :], cc[:, :])
                ot = vs if not S_BF16 else iop.tile([P, KW], F32, tag="ot")
                nc.vector.tensor_tensor(ot[:, :], vs[:, :], cc[:, :], AluOpType.mult)

                # ---- final blend: faulty (|x|>2.5) -> mean, else passthrough
                if _SELECT_BAND is not None:
                    nc.vector._custom_dve(
                        _SELECT_BAND,
                        out=ot[:, :],
                        in0=ot[:, :],
                        in1=xe[:, W : (K + 1) * W],
                        s0=2.5,
                        s1=-2.5,
                    )
                else:
                    kp = wk.tile([P, KW], mybir.dt.uint8, tag="kp")
                    if BITCAST_MASK:
                        nc.vector.tensor_scalar(
                            kp[:, :],
                            xe[:, W : (K + 1) * W].bitcast(I32),
                            SIGN_CLEAR,
                            ABS_BITS_25,
                            AluOpType.bitwise_and,
                            AluOpType.is_le,
                        )
                    else:
                        nc.vector.tensor_scalar(
                            kp[:, :],
                            xe[:, W : (K + 1) * W],
                            2.5,
                            None,
                            AluOpType.is_le,
                        )
                    nc.vector.copy_predicated(
                        ot[:, :], kp[:, :], xe[:, W : (K + 1) * W]
                    )

                nc.sync.dma_start(out[:, t * KW : (t + 1) * KW], ot[:, :])

    nc.compile()
    return nc


G = 8  # images per tile column in layout B
FB = G * W  # 2048 free elements per tile
NIG = P // G  # 16 image groups


def build_nc_b():
    nc = bacc.Bacc("TRN2", target_bir_lowering=False)
    _c = nc.alloc_sbuf_tensor("const-float32-2.5", [128, 1], F32)
    nc.gpsimd.memset(_c.ap(), 2.5)
    nc.const_aps.aps[(F32, 2.5)] = _c.ap()
    nc.all_engine_barrier()
    x = nc.dram_tensor("x", [P, FREE], F32, kind="ExternalInput")
    out = nc.dram_tensor("out", [P, FREE], F32, kind="ExternalOutput")
    # DRAM views with h on the partition axis: [h, image, w]
    xh = x.ap().rearrange("i (h w) -> h i w", w=W)
    oh = out.ap().rearrange("i (h w) -> h i w", w=W)

    with TileContext(nc) as tc:
        with (
            tc.tile_pool(name="const", bufs=1) as cp,
            tc.tile_pool(name="io", bufs=4) as iop,
            tc.tile_pool(name="wk", bufs=2) as wk,
            tc.tile_pool(name="ps", bufs=1, space="PSUM") as pp,
        ):
            # tridiagonal band of ones: band[k, m] = 1 iff |k - m| <= 1
            band_f = cp.tile([128, 128], F32, tag="bandf")
            nc.gpsimd.memset(band_f[:, :], 1.0)
            nc.gpsimd.affine_select(
                band_f[:, :], band_f[:, :],
                compare_op=AluOpType.is_ge, fill=0.0,
                base=1, pattern=[[1, 128]], channel_multiplier=-1,
            )
            nc.gpsimd.affine_select(
                band_f[:, :], band_f[:, :],
                compare_op=AluOpType.is_ge, fill=0.0,
                base=1, pattern=[[-1, 128]], channel_multiplier=1,
            )
            band_b = cp.tile([128, 128], BF16, tag="bandb")
            nc.vector.tensor_copy(band_b[:, :], band_f[:, :])
            # pick0[k, m] = 1 iff k==0, m==127  (block 0: out row 127 += next[0])
            # pick1[k, m] = 1 iff k==127, m==0  (block 1: out row 0 += prev[127])
            pick0_f = cp.tile([128, 128], F32, tag="p0f")
            nc.gpsimd.memset(pick0_f[:, :], 1.0)
            nc.gpsimd.affine_select(
                pick0_f[:, :], pick0_f[:, :],
                compare_op=AluOpType.is_ge, fill=0.0,
                base=-127, pattern=[[1, 128]], channel_multiplier=-1,
            )
            pick1_f = cp.tile([128, 128], F32, tag="p1f")
            nc.gpsimd.memset(pick1_f[:, :], 1.0)
            nc.gpsimd.affine_select(
                pick1_f[:, :], pick1_f[:, :],
                compare_op=AluOpType.is_ge, fill=0.0,
                base=-127, pattern=[[-1, 128]], channel_multiplier=1,
            )
            pick0_b = cp.tile([128, 128], BF16, tag="p0b")
            nc.vector.tensor_copy(pick0_b[:, :], pick0_f[:, :])
            pick1_b = cp.tile([128, 128], BF16, tag="p1b")
            nc.vector.tensor_copy(pick1_b[:, :], pick1_f[:, :])

            for ig in range(NIG):
                xb, vfb, csb, hsb = [], [], [], []
                for b in range(2):
                    t = iop.tile([P, FB], F32, tag="xb")
                    t3 = t[:, :].rearrange("p (g w) -> p g w", w=W)
                    nc.sync.dma_start(
                        t3, xh[b * 128 : (b + 1) * 128, ig * G : (ig + 1) * G, :]
                    )
                    xb.append(t)
                # vf = (|x| < 2.5) as 0/1, entirely on ScalarE
                for b in range(2):
                    ab = wk.tile([P, FB], F32, tag="ab")
                    nc.scalar.activation(
                        ab[:, :], xb[b][:, :], mybir.ActivationFunctionType.Abs
                    )
                    nc.scalar.activation(
                        ab[:, :], ab[:, :],
                        mybir.ActivationFunctionType.Relu, bias=2.5, scale=-1.0,
                    )
                    vf = iop.tile([P, FB], BF16, tag="vf")
                    nc.scalar.activation(
                        vf[:, :], ab[:, :],
                        mybir.ActivationFunctionType.Tanh, scale=1.0e38,
                    )
                    vfb.append(vf)
                # horizontal 3-tap count sums (bf16, 2x mode)
                for b in range(2):
                    vf = vfb[b]
                    cs = iop.tile([P, FB], BF16, tag="cs")
                    nc.vector.tensor_tensor(
                        cs[:, 1 : FB - 1], vf[:, 0 : FB - 2], vf[:, 1 : FB - 1],
                        AluOpType.add,
                    )
                    nc.vector.tensor_tensor(
                        cs[:, 1 : FB - 1], cs[:, 1 : FB - 1], vf[:, 2:FB],
                        AluOpType.add,
                    )
                    cs3 = cs[:, :].rearrange("p (g w) -> p g w", w=W)
                    vf3 = vf[:, :].rearrange("p (g w) -> p g w", w=W)
                    nc.vector.tensor_tensor(
                        cs3[:, :, 0:1], vf3[:, :, 0:1], vf3[:, :, 1:2], AluOpType.add
                    )
                    nc.vector.tensor_tensor(
                        cs3[:, :, W - 1 : W], vf3[:, :, W - 2 : W - 1],
                        vf3[:, :, W - 1 : W], AluOpType.add,
                    )
                    csb.append(cs)
                # xv = x * vf and horizontal 3-tap value sums (fp32)
                for b in range(2):
                    xv = wk.tile([P, FB], F32, tag="xv")
                    nc.vector.tensor_tensor(
                        xv[:, :], xb[b][:, :], vfb[b][:, :], AluOpType.mult
                    )
                    hs = iop.tile([P, FB], F32, tag="hs")
                    nc.vector.tensor_tensor(
                        hs[:, 1 : FB - 1], xv[:, 0 : FB - 2], xv[:, 1 : FB - 1],
                        AluOpType.add,
                    )
                    nc.vector.tensor_tensor(
                        hs[:, 1 : FB - 1], hs[:, 1 : FB - 1], xv[:, 2:FB],
                        AluOpType.add,
                    )
                    hs3 = hs[:, :].rearrange("p (g w) -> p g w", w=W)
                    xv3 = xv[:, :].rearrange("p (g w) -> p g w", w=W)
                    nc.vector.tensor_tensor(
                        hs3[:, :, 0:1], xv3[:, :, 0:1], xv3[:, :, 1:2], AluOpType.add
                    )
                    nc.vector.tensor_tensor(
                        hs3[:, :, W - 1 : W], xv3[:, :, W - 2 : W - 1],
                        xv3[:, :, W - 1 : W], AluOpType.add,
                    )
                    hsb.append(hs)
                # vertical 3-tap sums on the TensorEngine (band matmuls),
                # block-boundary rows patched with one-row DVE adds
                for b in range(2):
                    other = 1 - b
                    pick_b16 = pick0_b if b == 0 else pick1_b
                    pick_f32 = pick0_f if b == 0 else pick1_f
                    ps_c = pp.tile([P, FB], F32, tag="psc")
                    for j in range(FB // 512):
                        sl = slice(j * 512, (j + 1) * 512)
                        nc.tensor.matmul(
                            ps_c[:, sl], band_b[:, :], csb[b][:, sl],
                            start=True, stop=False,
                        )
                        nc.tensor.matmul(
                            ps_c[:, sl], pick_b16[:, :], csb[other][:, sl],
                            start=False, stop=True,
                        )
                    ps_s = pp.tile([P, FB], F32, tag="pss")
                    for j in range(FB // 512):
                        sl = slice(j * 512, (j + 1) * 512)
                        nc.tensor.matmul(
                            ps_s[:, sl], band_f[:, :], hsb[b][:, sl],
                            start=True, stop=False,
                        )
                        nc.tensor.matmul(
                            ps_s[:, sl], pick_f32[:, :], hsb[other][:, sl],
                            start=False, stop=True,
                        )
                    r = wk.tile([P, FB], F32, tag="r")
                    nc.vector.reciprocal_approx_fast(r[:, :], ps_c[:, :])
                    ot = wk.tile([P, FB], F32, tag="ot")
                    nc.vector.tensor_tensor(
                        ot[:, :], ps_s[:, :], r[:, :], AluOpType.mult
                    )
                    nc.vector._custom_dve(
                        _SELECT_BAND,
                        out=ot[:, :], in0=ot[:, :], in1=xb[b][:, :],
                        s0=2.5, s1=-2.5,
                    )
                    ot3 = ot[:, :].rearrange("p (g w) -> p g w", w=W)
                    nc.sync.dma_start(
                        oh[b * 128 : (b + 1) * 128, ig * G : (ig + 1) * G, :], ot3
                    )

    nc.compile()
    return nc


def _get_nc():
    global _NC_CACHE
    if _NC_CACHE is None:
        _NC_CACHE = build_nc_b() if LAYOUT == "B" else build_nc()
    return _NC_CACHE


def kernel(x: np.ndarray) -> np.ndarray:
    assert x.shape == (N, C, H, W) and x.dtype == np.float32
    xs = np.ascontiguousarray(x).reshape(NCORES, P, FREE)
    in_maps = [{"x": xs[i]} for i in range(NCORES)]
    res = bass_utils.run_bass_kernel_spmd(
        _get_nc(), in_maps, core_ids=list(range(NCORES))
    )
    out = np.stack([res.results[i]["out"] for i in range(NCORES)])
    return out.reshape(N, C, H, W)
